# revision 79
# baseline (speedup 1.0000x reference)
"""Bass/Tile TRN2 kernel for nn_BayesHead (projected single-head attention,
near-causal mask tril(diag=1), double 1/sqrt(64) scaling).

Strategy (8 NeuronCores, pure data-parallel SPMD — no collectives):
  - core j handles batch b = j//2 with key-parity p = j%2.
  - Each core projects ALL 4096 queries of its batch, and its HALF of the
    keys/values (interleaved 128-row blocks: global block g = 2*sigma + p).
  - Flash-style partial softmax without max-subtraction (scores are in
    [-1,1] after the 1/64 scaling, so exp is safe): each core produces
    O_p[h, t] = sum_{s in its keys, s <= t+1} exp(S) * V[s, h] plus a
    denominator row (ones-column trick).  The host sums the two partials
    per batch and normalizes.

v3 scheduling (vs the ~109-127us v2 baseline; measured ~95-105us):
  - Only 3 distinct mask tensors exist (thr depends only on 2s-4i in
    {0,2,4}); built once on the DVE instead of 23 per-tile builds.
  - Singleton s-tiles trimmed to cols [480,512) (p0 singleton has exactly
    1 live column; p1's is fully dead).
  - DMA stream: weights merged into one transfer (w3), iota|ident|thr into
    another (misc); 18 issues total, deadline-ordered with k0 first (the
    HWDGE ring serializes transfers at ~2.4us/MB, so issue order == arrival
    order); output tiles go out through the gpsimd SWDGE ring to keep the
    input ring free.
  - PE warm-up matmuls during the DMA-dead start so HAM is at K=8/8
    (2.4 GHz) when the first projection lands.
  - Projections run inside each attention's mid-hook (between the last
    score group and the PV flush) so the PE chews them while the ACT
    engine drains that tile's exp stream; per-attention PV pipeline with
    LAG=3.
  - NOTE (measured): col-tiled matmul pairs at (0,0)/(0,64) run
    CONCURRENTLY only when both stream the SAME moving operand (the dup
    projections exploit this); with different moving operands they
    serialize.  Row-tiled score pairs with different moving operands DO
    run concurrently.
"""

import numpy as np
from contextlib import ExitStack

import concourse.bass as bass
import concourse.mybir as mybir
import concourse.tile as tile
from concourse import bacc
from concourse.bass import ts
from concourse.bass_utils import run_bass_kernel_spmd

B, T, C, H = 4, 4096, 1024, 64
NCORES = 8
TQ = 512                       # query-tile width
NQT = T // TQ                  # 8 query tiles
NSB = (T // 2) // 128          # 16 local key tiles (128 rows each)
NCT = C // 128                 # 8 contraction tiles
TH = T // 2
# s-tile capacity per query tile (identical for both parities; covers causal
# reach ceil((4i+5)/2), capped at the 16 local tiles)
CAPS = [min(NSB, 2 * i + 3) for i in range(NQT)]
MASK_FROM = [2 * i for i in range(NQT)]  # sigma >= 2i may cross the diagonal
# The mask for tile (i, s) depends only on e = 2s - 4i in {0, 2, 4}:
# thr = 128*(2s+p) + r - 512i - 1 = 128*e + 128*p + r - 1.  Three masks total.
M_IDX = {(i, s): (2 * s - 4 * i) // 2
         for i in range(NQT) for s in range(MASK_FROM[i], CAPS[i])}
N_MASKED = 3
W0 = 480                       # live-column window start for singleton s-tiles
# (p0 singleton has exactly 1 live col (511); p1 singleton is fully dead)
FP = mybir.dt.float16
F32 = mybir.dt.float32
SCALE = 1.0 / H                # (H**-0.5) applied twice


def build_bass():
    nc = bacc.Bacc("TRN2", target_bir_lowering=False, num_devices=NCORES)
    # DRAM layouts are pre-transposed on host and chunk-major:
    # x[p, chunk, ct, col] = x.T[128*ct+p, 512*chunk+col], so each 512-col
    # chunk DMA moves 8KB contiguous per partition (128 fat descriptors)
    qT = nc.declare_dram_parameter("qT", [128, NQT, NCT, 512], FP, isOutput=False)
    kT = nc.declare_dram_parameter("kT", [128, NQT // 2, NCT, 512], FP, isOutput=False)
    vT = nc.declare_dram_parameter("vT", [128, NQT // 2, NCT, 512], FP, isOutput=False)
    # wq|wk|wv merged into one DMA; iota|ident|thr merged into one (all fp16)
    w3 = nc.declare_dram_parameter("w3", [128, 3, NCT, H], FP, isOutput=False)
    misc = nc.declare_dram_parameter("misc", [128, TQ + 64 + N_MASKED], FP,
                                     isOutput=False)
    out = nc.declare_dram_parameter("out", [H + 1, T], F32, isOutput=True)

    with ExitStack() as ctx:
        tc = ctx.enter_context(tile.TileContext(nc))
        singles = ctx.enter_context(tc.tile_pool(name="singles", bufs=1))
        pt_pool = ctx.enter_context(tc.tile_pool(name="pt", bufs=7))
        outsb_pool = ctx.enter_context(tc.tile_pool(name="outsb", bufs=6))
        stage_pool = ctx.enter_context(tc.tile_pool(name="stage", bufs=2))
        psum_s = ctx.enter_context(tc.tile_pool(name="psum_s", bufs=3, space="PSUM"))
        psum_o = ctx.enter_context(tc.tile_pool(name="psum_o", bufs=2, space="PSUM"))

        # SBUF-resident tiles
        misc_sb = singles.tile([128, TQ + 64 + N_MASKED], FP)
        iota_sb = misc_sb[:, 0:TQ]
        id_sb = misc_sb[0:64, TQ:TQ + 64]
        thr_sb = singles.tile([128, N_MASKED], F32)
        w3_sb = singles.tile([128, 3, NCT, H], FP)
        wq_sb = w3_sb[:, 0]
        wk_sb = w3_sb[:, 1]
        wv_sb = w3_sb[:, 2]
        q_sb = singles.tile([128, NQT, NCT, 512], FP)
        k_sb = singles.tile([128, NQT // 2, NCT, 512], FP)
        v_sb = singles.tile([128, NQT // 2, NCT, 512], FP)

        qp_sb = singles.tile([128, T], FP)        # Q^T [h, t], dup on parts 64-127
        kp_sb = singles.tile([128, TH], FP)       # K^T [h, s], dup on parts 64-127
        va_sb = singles.tile([128, NSB, H + 1], FP)  # V rows [s, h] + ones col
        masks_sb = singles.tile([128, N_MASKED, TQ], FP)

        # ---- DMA issue stream (sync engine), deadline order ----
        # Arrival pacing is ~2.85us/MB; the exp (ACT) stream is paced by q_i
        # arrivals early on, so q chunks go as early as k/v deadlines allow.
        def dq(c):
            nc.sync.dma_start(out=q_sb[:, c, :, :], in_=qT[:, c, :, :])

        def dk(c, c0=0, c1=512):
            nc.sync.dma_start(out=k_sb[:, c, :, c0:c1], in_=kT[:, c, :, c0:c1])

        def dv(c, c0=0, c1=512):
            nc.sync.dma_start(out=v_sb[:, c, :, c0:c1], in_=vT[:, c, :, c0:c1])

        nc.sync.dma_start(out=w3_sb, in_=w3[:, :, :, :])
        dk(0)
        dq(0)
        dq(1)
        nc.sync.dma_start(out=misc_sb, in_=misc[:, :])
        dv(0)
        dq(2)
        dk(1)
        dq(3)
        dv(1)
        dq(4)
        dk(2)
        dq(5)
        dk(3)
        dq(6)
        dq(7)
        # v2/v3 ride the gpsimd SWDGE ring (issued after out(0)/out(1) in
        # finalize()) so the sync ring's q-chunk arrivals come ~5us earlier

        # ones column for the softmax denominator
        nc.vector.memset(va_sb[:, :, H:H + 1], 1.0)

        # PE warm-up: the first ~10us are DMA-dead (preamble + q0/k0 landing);
        # keep the PE busy on junk matmuls so HAM reaches K=8/8 (2.4 GHz)
        # before the first projection instead of ramping mid-projection.
        WARM = 14
        if WARM:
            warm_sb = singles.tile([128, 576], FP)
            nc.vector.memset(warm_sb, 0.25)
            warm_ps = psum_s.tile([128, 512], F32, tag="ps")
            for _ in range(WARM):
                nc.tensor.matmul(warm_ps, warm_sb[:, 0:128],
                                 warm_sb[:, 64:576],
                                 start=True, stop=True, skip_group_check=True)



        def build_masks():
            # Only 3 distinct masks exist (e = 2s-4i in {0,2,4}); build once.
            nc.vector.tensor_copy(            # fp16 -> f32 (is_ge wants f32)
                thr_sb, misc_sb[:, TQ + 64:TQ + 64 + N_MASKED])
            for m in range(N_MASKED):
                nc.vector.tensor_scalar(
                    masks_sb[:, m, :], iota_sb[:, :], thr_sb[:, m:m + 1],
                    None, mybir.AluOpType.is_ge)

        def q_proj(tq):
            pq = psum_s.tile([128, 512], F32, tag="ps")
            for ct in range(NCT):
                nc.tensor.matmul(pq[0:64, :], wq_sb[:, ct, :],
                                 q_sb[:, tq, ct, :], tile_position=(0, 0),
                                 start=(ct == 0), stop=(ct == NCT - 1))
                nc.tensor.matmul(pq[64:128, :], wq_sb[:, ct, :],
                                 q_sb[:, tq, ct, :], tile_position=(0, 64),
                                 start=(ct == 0), stop=(ct == NCT - 1),
                                 skip_group_check=True)
            nc.vector.tensor_copy(qp_sb[:, ts(tq, 512)], pq)

        def k_proj(c4, c0=0, c1=512):
            pk = psum_s.tile([128, 512], F32, tag="ps")
            for ct in range(NCT):
                nc.tensor.matmul(pk[0:64, c0:c1], wk_sb[:, ct, :],
                                 k_sb[:, c4, ct, c0:c1], tile_position=(0, 0),
                                 start=(ct == 0), stop=(ct == NCT - 1))
                nc.tensor.matmul(pk[64:128, c0:c1], wk_sb[:, ct, :],
                                 k_sb[:, c4, ct, c0:c1], tile_position=(0, 64),
                                 start=(ct == 0), stop=(ct == NCT - 1),
                                 skip_group_check=True)
            nc.vector.tensor_copy(kp_sb[:, 512 * c4 + c0:512 * c4 + c1],
                                  pk[:, c0:c1])

        def v_proj(c4, j0=0, j1=4):
            cols = slice(128 * j0, 128 * j1)
            pv = psum_s.tile([64, 512], F32, tag="ps")
            for ct in range(NCT):
                nc.tensor.matmul(pv[:, cols], wv_sb[:, ct, :],
                                 v_sb[:, c4, ct, cols],
                                 start=(ct == 0), stop=(ct == NCT - 1))
            vt_stage = stage_pool.tile([64, 512], FP)
            nc.vector.tensor_copy(vt_stage[:, cols], pv[:, cols])
            for j in range(j0, j1):
                sig = c4 * 4 + j
                ptr = psum_o.tile([128, H], FP, tag="oacc")
                nc.tensor.transpose(ptr, vt_stage[:, ts(j, 128)], id_sb)
                nc.vector.tensor_copy(va_sb[:, sig, 0:H], ptr)

        def emit_s(i, kind, g0):
            # scores matmuls + exp + mask for one group of tile i; returns pt
            if kind == "pair":
                ps = psum_s.tile([128, 1024], F32, tag="ps")
                for g in (0, 1):
                    sig = g0 + g
                    nc.tensor.matmul(ps[:, ts(g, 512)],
                                     kp_sb[ts(g, 64), ts(sig, 128)],
                                     qp_sb[ts(g, 64), ts(i, 512)],
                                     tile_position=(64 * g, 0),
                                     start=True, stop=True)
                pt = pt_pool.tile([128, 1024], FP)
                nc.scalar.activation(pt, ps,
                                     mybir.ActivationFunctionType.Exp,
                                     scale=SCALE)
                for g in (0, 1):
                    sig = g0 + g
                    if sig >= MASK_FROM[i]:
                        m = M_IDX[(i, sig)]
                        nc.vector.tensor_mul(pt[:, ts(g, 512)],
                                             pt[:, ts(g, 512)],
                                             masks_sb[:, m, :])
            else:  # singleton: full-width scores (PSUM zero-region rule),
                # but exp/mask/PV trimmed to the live cols [W0:512)
                sig = g0
                m = M_IDX[(i, sig)]
                ps = psum_s.tile([128, 512], F32, tag="ps")
                nc.tensor.matmul(ps,
                                 kp_sb[0:64, ts(sig, 128)],
                                 qp_sb[0:64, ts(i, 512)],
                                 tile_position=(0, 0), start=True, stop=True)
                pt = pt_pool.tile([128, 512], FP)
                nc.scalar.activation(pt[:, W0:512], ps[:, W0:512],
                                     mybir.ActivationFunctionType.Exp,
                                     scale=SCALE)
                nc.vector.tensor_mul(pt[:, W0:512], pt[:, W0:512],
                                     masks_sb[:, m, W0:512])
            return pt

        def groups_of(i):
            # leading full pairs, then (i<7) the nearly-dead singleton
            # (trimmed to cols [W0,512)), then the diagonal pair (2i, 2i+1)
            # last so the accumulation stop lands on a full-width matmul.
            cap = CAPS[i]
            groups = []
            lead = cap if i == 7 else 2 * i
            for g0 in range(0, lead, 2):
                groups.append(("pair", g0))
            if i == 0:
                # the first PV writing po must be full width (PSUM zero-region
                # start semantics), so the trimmed singleton goes last
                groups = [("pair", 0), ("single", 2)]
            elif i < 7:
                groups.append(("single", cap - 1))
                groups.append(("pair", 2 * i))
            return groups

        PEELED = {}                # tile -> pre-emitted leading score groups
        PEEL = 1                   # max lead pairs peeled across the boundary

        def attention(i, mid_q=None, mid=None, peel_next=False):
            po = psum_o.tile([H + 1, 512], F32, tag="oacc")
            groups = groups_of(i)

            state = {"first": True}

            def emit_pv(kind, g0, pt, last_grp):
                if kind == "pair":
                    for g in (0, 1):
                        sig = g0 + g
                        nc.tensor.matmul(po, va_sb[:, sig, :], pt[:, ts(g, 512)],
                                         start=state["first"],
                                         stop=(last_grp and g == 1))
                        state["first"] = False
                else:
                    nc.tensor.matmul(po[:, W0:512], va_sb[:, g0, :],
                                     pt[:, W0:512], start=state["first"],
                                     stop=last_grp)
                    state["first"] = False

            def finalize():
                osb = outsb_pool.tile([H + 1, 512], F32)
                nc.vector.tensor_copy(osb, po)
                # gpsimd SWDGE ring: keeps the sync HWDGE ring free for the
                # input stream (ring-full blocking serializes transfers)
                nc.gpsimd.dma_start(out=out[:, ts(i, 512)], in_=osb)
                if i < 2:
                    # late v chunks on the SWDGE ring, throttled behind the
                    # first outs so they don't steal bandwidth from the
                    # early q/k stream; v2 lands ~30 (needed ~31 in mid(2)),
                    # v3 ~35 (needed ~43 in mid(4))
                    c = i + 2
                    nc.gpsimd.dma_start(out=v_sb[:, c, :, :],
                                        in_=vT[:, c, :, :])

            # software pipeline: PV stream lags the scores stream by LAG
            # groups so the PE never stalls on the exp+mask latency; tail
            # PVs flush after mid() (the next tile's projections), by which
            # time their exps are long done.
            LAG = 3
            pend = []
            start_gi = 0
            if i in PEELED:        # leading groups emitted by the previous
                pend.extend(PEELED.pop(i))  # attention's peel
                start_gi = len(pend)
            for gi, (kind, g0) in enumerate(groups):
                if gi < start_gi:
                    continue
                if gi == len(groups) - 1 and mid_q is not None:
                    # next tile's q-projection BEFORE the last score group:
                    # its DVE evacuation then precedes this group's mask
                    # multiply in the DVE queue, so the peeled scores below
                    # aren't stalled behind an ACT-gated mask op
                    mid_q()
                    mid_q = None
                pt = emit_s(i, kind, g0)
                pend.append((kind, g0, pt))
                if len(pend) > LAG:
                    k_, g_, pt_ = pend.pop(0)
                    emit_pv(k_, g_, pt_, last_grp=False)
            if mid_q is not None:
                mid_q()
            if peel_next:
                # pre-emit the NEXT tile's leading score PAIRS so the ACT
                # engine rolls straight into them the moment this tile's exps
                # drain (instead of waiting for this tile's tail-PV flush or
                # the k/v projections in mid()).  Only lead pairs are safe to
                # peel: they need kp tiles projected attentions ago, while
                # the singleton/diag need the k-chunk projected in mid().
                nxt = groups_of(i + 1)
                n_lead = sum(1 for k_, g_ in nxt
                             if k_ == "pair" and g_ < 2 * (i + 1))
                PEELED[i + 1] = [(k_, g_, emit_s(i + 1, k_, g_))
                                 for k_, g_ in nxt[:min(PEEL, max(n_lead, 1))]]
            if mid is not None:
                # next tiles' k/v projections: the PE chews them while the
                # ACT engine runs this tile's trailing exps + the peel
                mid()
            for j, (k_, g_, pt_) in enumerate(pend):
                emit_pv(k_, g_, pt_, last_grp=(j == len(pend) - 1))
            finalize()

        # ---- compute schedule: deadline-aligned with the DMA stream ----
        # attention(i) consumes qp_i, kp s-tiles <= 2i+2, va s-tiles <= 2i+2.
        # Each attention's mid-hook carries upcoming projections so they
        # overlap that tile's trailing exp stream on the ACT engine.
        build_masks()
        k_proj(0)                  # s0-3
        q_proj(0)
        v_proj(0)                  # va 0-3
        attention(0, mid_q=lambda: q_proj(1),
                  mid=lambda: (k_proj(1), v_proj(1, 0, 2)), peel_next=True)
        attention(1, mid_q=lambda: q_proj(2),
                  mid=lambda: v_proj(1, 2, 4), peel_next=True)
        attention(2, mid_q=lambda: q_proj(3),
                  mid=lambda: (k_proj(2), v_proj(2, 0, 2)), peel_next=True)
        attention(3, mid_q=lambda: q_proj(4),
                  mid=lambda: v_proj(2, 2, 4), peel_next=True)
        attention(4, mid_q=lambda: q_proj(5),
                  mid=lambda: (k_proj(3), v_proj(3, 0, 2)), peel_next=True)
        attention(5, mid_q=lambda: q_proj(6),
                  mid=lambda: v_proj(3, 2, 4), peel_next=True)
        attention(6, mid_q=lambda: q_proj(7), peel_next=True)
        attention(7)

    nc.compile()
    return nc


_NC = None


def _get_nc():
    global _NC
    if _NC is None:
        _NC = build_bass()
    return _NC


def _prep_core_inputs(q, k, v, Wq, Wk, Wv):
    f2 = np.float16

    def wprep(W):
        # SBUF layout [p, ct, h] = W.T[ct*128+p, h]
        return np.ascontiguousarray(W.T.reshape(NCT, 128, H).transpose(1, 0, 2)).astype(f2)

    def xprep(x):
        # [p, chunk, ct, col] = x.T[128*ct+p, 512*chunk+col]
        xt = x.T.astype(f2)                       # [C, T']
        nch = xt.shape[1] // 512
        return np.ascontiguousarray(
            xt.reshape(NCT, 128, nch, 512).transpose(1, 2, 0, 3))

    w3_h = np.ascontiguousarray(
        np.stack([wprep(Wq), wprep(Wk), wprep(Wv)], axis=1))

    r = np.arange(128)
    in_maps = []
    for j in range(NCORES):
        b, p = j // 2, j % 2
        rows = (np.arange(TH) // 128) * 256 + p * 128 + (np.arange(TH) % 128)
        qT_h = xprep(q[b])
        kT_h = xprep(k[b][rows])
        vT_h = xprep(v[b][rows])
        misc_h = np.zeros((128, TQ + 64 + N_MASKED), f2)
        misc_h[:, 0:TQ] = np.arange(TQ, dtype=np.float32)[None, :]
        misc_h[0:64, TQ:TQ + 64] = np.eye(64, dtype=f2)
        for m in range(N_MASKED):
            misc_h[:, TQ + 64 + m] = (256 * m + 128 * p + r - 1).astype(f2)
        in_maps.append({
            "qT": qT_h, "kT": kT_h, "vT": vT_h,
            "w3": w3_h, "misc": misc_h,
        })
    return in_maps


def _run(inputs, trace=False, trace_kwargs=None):
    nc = _get_nc()
    in_maps = _prep_core_inputs(
        inputs["q"], inputs["k"], inputs["v"],
        inputs["Wq"], inputs["Wk"], inputs["Wv"])
    res = run_bass_kernel_spmd(nc, in_maps, list(range(NCORES)), trace=trace,
                               **(trace_kwargs or {}))
    outs = [res.results[j]["out"] for j in range(NCORES)]
    y = np.empty((B, T, H), np.float32)
    for b in range(B):
        s = outs[2 * b] + outs[2 * b + 1]      # [H+1, T]
        y[b] = (s[:H] / s[H:H + 1]).T
    return y, res


def kernel(q, k, v, Wq, Wk, Wv):
    y, _ = _run({"q": np.asarray(q), "k": np.asarray(k), "v": np.asarray(v),
                 "Wq": np.asarray(Wq), "Wk": np.asarray(Wk), "Wv": np.asarray(Wv)})
    return y



# revision 81
# speedup vs baseline: 1.1156x; 1.1156x over previous
"""Bass/Tile TRN2 kernel for nn_BayesHead (projected single-head attention,
near-causal mask tril(diag=1), double 1/sqrt(64) scaling).

Strategy (8 NeuronCores, pure data-parallel SPMD — no collectives):
  - core j handles batch b = j//2 with key-parity p = j%2.
  - Each core projects ALL 4096 queries of its batch, and its HALF of the
    keys/values (interleaved 128-row blocks: global block g = 2*sigma + p).
  - Flash-style partial softmax without max-subtraction (scores are in
    [-1,1] after the 1/64 scaling, so exp is safe): each core produces
    O_p[h, t] = sum_{s in its keys, s <= t+1} exp(S) * V[s, h] plus a
    denominator row (ones-column trick).  The host sums the two partials
    per batch and normalizes.

v3 scheduling (vs the ~109-127us v2 baseline; measured ~95-105us):
  - Only 3 distinct mask tensors exist (thr depends only on 2s-4i in
    {0,2,4}); built once on the DVE instead of 23 per-tile builds.
  - Singleton s-tiles trimmed to cols [480,512) (p0 singleton has exactly
    1 live column; p1's is fully dead).
  - DMA stream: weights merged into one transfer (w3), iota|ident|thr into
    another (misc); 18 issues total, deadline-ordered with k0 first (the
    HWDGE ring serializes transfers at ~2.4us/MB, so issue order == arrival
    order); output tiles go out through the gpsimd SWDGE ring to keep the
    input ring free.
  - PE warm-up matmuls during the DMA-dead start so HAM is at K=8/8
    (2.4 GHz) when the first projection lands.
  - Projections run inside each attention's mid-hook (between the last
    score group and the PV flush) so the PE chews them while the ACT
    engine drains that tile's exp stream; per-attention PV pipeline with
    LAG=3.
  - NOTE (measured): col-tiled matmul pairs at (0,0)/(0,64) run
    CONCURRENTLY only when both stream the SAME moving operand (the dup
    projections exploit this); with different moving operands they
    serialize.  Row-tiled score pairs with different moving operands DO
    run concurrently.
"""

import numpy as np
from contextlib import ExitStack

import concourse.bass as bass
import concourse.mybir as mybir
import concourse.tile as tile
from concourse import bacc
from concourse.bass import ts
from concourse.bass_utils import run_bass_kernel_spmd

B, T, C, H = 4, 4096, 1024, 64
NCORES = 8
TQ = 512                       # query-tile width
NQT = T // TQ                  # 8 query tiles
NSB = (T // 2) // 128          # 16 local key tiles (128 rows each)
NCT = C // 128                 # 8 contraction tiles
TH = T // 2
# s-tile capacity per query tile (identical for both parities; covers causal
# reach ceil((4i+5)/2), capped at the 16 local tiles)
CAPS = [min(NSB, 2 * i + 3) for i in range(NQT)]
MASK_FROM = [2 * i for i in range(NQT)]  # sigma >= 2i may cross the diagonal
# The mask for tile (i, s) depends only on e = 2s - 4i in {0, 2, 4}:
# thr = 128*(2s+p) + r - 512i - 1 = 128*e + 128*p + r - 1.  Three masks total.
M_IDX = {(i, s): (2 * s - 4 * i) // 2
         for i in range(NQT) for s in range(MASK_FROM[i], CAPS[i])}
N_MASKED = 3
W0 = 480                       # live-column window start for singleton s-tiles
# (p0 singleton has exactly 1 live col (511); p1 singleton is fully dead)
FP = mybir.dt.float16
F32 = mybir.dt.float32
SCALE = 1.0 / H                # (H**-0.5) applied twice


def build_bass():
    nc = bacc.Bacc("TRN2", target_bir_lowering=False, num_devices=NCORES)
    # DRAM layouts are pre-transposed on host and chunk-major:
    # x[p, chunk, ct, col] = x.T[128*ct+p, 512*chunk+col], so each 512-col
    # chunk DMA moves 8KB contiguous per partition (128 fat descriptors)
    qT = nc.declare_dram_parameter("qT", [128, NQT, NCT, 512], FP, isOutput=False)
    kT = nc.declare_dram_parameter("kT", [128, NQT // 2, NCT, 512], FP, isOutput=False)
    vT = nc.declare_dram_parameter("vT", [128, NQT // 2, NCT, 512], FP, isOutput=False)
    # wq|wk|wv merged into one DMA; iota|ident|thr merged into one (all fp16)
    w3 = nc.declare_dram_parameter("w3", [128, 3, NCT, H], FP, isOutput=False)
    misc = nc.declare_dram_parameter("misc", [128, TQ + 64 + N_MASKED], FP,
                                     isOutput=False)
    out = nc.declare_dram_parameter("out", [H + 1, T], F32, isOutput=True)

    with ExitStack() as ctx:
        tc = ctx.enter_context(tile.TileContext(nc))
        singles = ctx.enter_context(tc.tile_pool(name="singles", bufs=1))
        pt_pool = ctx.enter_context(tc.tile_pool(name="pt", bufs=7))
        outsb_pool = ctx.enter_context(tc.tile_pool(name="outsb", bufs=6))
        stage_pool = ctx.enter_context(tc.tile_pool(name="stage", bufs=2))
        psum_s = ctx.enter_context(tc.tile_pool(name="psum_s", bufs=3, space="PSUM"))
        psum_o = ctx.enter_context(tc.tile_pool(name="psum_o", bufs=2, space="PSUM"))

        # SBUF-resident tiles
        misc_sb = singles.tile([128, TQ + 64 + N_MASKED], FP)
        iota_sb = misc_sb[:, 0:TQ]
        id_sb = misc_sb[0:64, TQ:TQ + 64]
        thr_sb = singles.tile([128, N_MASKED], F32)
        w3_sb = singles.tile([128, 3, NCT, H], FP)
        wq_sb = w3_sb[:, 0]
        wk_sb = w3_sb[:, 1]
        wv_sb = w3_sb[:, 2]
        q_sb = singles.tile([128, NQT, NCT, 512], FP)
        k_sb = singles.tile([128, NQT // 2, NCT, 512], FP)
        v_sb = singles.tile([128, NQT // 2, NCT, 512], FP)

        qp_sb = singles.tile([128, T], FP)        # Q^T [h, t], dup on parts 64-127
        kp_sb = singles.tile([128, TH], FP)       # K^T [h, s], dup on parts 64-127
        va_sb = singles.tile([128, NSB, H + 1], FP)  # V rows [s, h] + ones col
        masks_sb = singles.tile([128, N_MASKED, TQ], FP)

        # ---- DMA issue stream (sync engine), deadline order ----
        # Arrival pacing is ~2.85us/MB; the exp (ACT) stream is paced by q_i
        # arrivals early on, so q chunks go as early as k/v deadlines allow.
        def dq(c):
            nc.sync.dma_start(out=q_sb[:, c, :, :], in_=qT[:, c, :, :])

        def dk(c, c0=0, c1=512):
            nc.sync.dma_start(out=k_sb[:, c, :, c0:c1], in_=kT[:, c, :, c0:c1])

        def dv(c, c0=0, c1=512):
            nc.sync.dma_start(out=v_sb[:, c, :, c0:c1], in_=vT[:, c, :, c0:c1])

        nc.sync.dma_start(out=w3_sb, in_=w3[:, :, :, :])
        dk(0)
        dq(0)
        dq(1)
        nc.sync.dma_start(out=misc_sb, in_=misc[:, :])
        dv(0)
        dq(2)
        dk(1)
        dq(3)
        dv(1)
        dq(4)
        dk(2)
        dq(5)
        dv(2)
        dk(3)
        dq(6)
        dq(7)
        dv(3)

        # ones column for the softmax denominator
        nc.vector.memset(va_sb[:, :, H:H + 1], 1.0)

        # PE warm-up: the first ~10us are DMA-dead (preamble + q0/k0 landing);
        # keep the PE busy on junk matmuls so HAM reaches K=8/8 (2.4 GHz)
        # before the first projection instead of ramping mid-projection.
        WARM = 14
        if WARM:
            warm_sb = singles.tile([128, 576], FP)
            nc.vector.memset(warm_sb, 0.25)
            warm_ps = psum_s.tile([128, 512], F32, tag="ps")
            for _ in range(WARM):
                nc.tensor.matmul(warm_ps, warm_sb[:, 0:128],
                                 warm_sb[:, 64:576],
                                 start=True, stop=True, skip_group_check=True)



        def build_masks():
            # Only 3 distinct masks exist (e = 2s-4i in {0,2,4}); build once.
            nc.vector.tensor_copy(            # fp16 -> f32 (is_ge wants f32)
                thr_sb, misc_sb[:, TQ + 64:TQ + 64 + N_MASKED])
            for m in range(N_MASKED):
                nc.vector.tensor_scalar(
                    masks_sb[:, m, :], iota_sb[:, :], thr_sb[:, m:m + 1],
                    None, mybir.AluOpType.is_ge)

        def q_proj(tq):
            pq = psum_s.tile([128, 512], F32, tag="ps")
            for ct in range(NCT):
                nc.tensor.matmul(pq[0:64, :], wq_sb[:, ct, :],
                                 q_sb[:, tq, ct, :], tile_position=(0, 0),
                                 start=(ct == 0), stop=(ct == NCT - 1))
                nc.tensor.matmul(pq[64:128, :], wq_sb[:, ct, :],
                                 q_sb[:, tq, ct, :], tile_position=(0, 64),
                                 start=(ct == 0), stop=(ct == NCT - 1),
                                 skip_group_check=True)
            nc.vector.tensor_copy(qp_sb[:, ts(tq, 512)], pq)

        def k_proj(c4, c0=0, c1=512):
            pk = psum_s.tile([128, 512], F32, tag="ps")
            for ct in range(NCT):
                nc.tensor.matmul(pk[0:64, c0:c1], wk_sb[:, ct, :],
                                 k_sb[:, c4, ct, c0:c1], tile_position=(0, 0),
                                 start=(ct == 0), stop=(ct == NCT - 1))
                nc.tensor.matmul(pk[64:128, c0:c1], wk_sb[:, ct, :],
                                 k_sb[:, c4, ct, c0:c1], tile_position=(0, 64),
                                 start=(ct == 0), stop=(ct == NCT - 1),
                                 skip_group_check=True)
            nc.vector.tensor_copy(kp_sb[:, 512 * c4 + c0:512 * c4 + c1],
                                  pk[:, c0:c1])

        def v_proj(c4, j0=0, j1=4):
            cols = slice(128 * j0, 128 * j1)
            pv = psum_s.tile([64, 512], F32, tag="ps")
            for ct in range(NCT):
                nc.tensor.matmul(pv[:, cols], wv_sb[:, ct, :],
                                 v_sb[:, c4, ct, cols],
                                 start=(ct == 0), stop=(ct == NCT - 1))
            vt_stage = stage_pool.tile([64, 512], FP)
            nc.vector.tensor_copy(vt_stage[:, cols], pv[:, cols])
            for j in range(j0, j1):
                sig = c4 * 4 + j
                ptr = psum_o.tile([128, H], FP, tag="oacc")
                nc.tensor.transpose(ptr, vt_stage[:, ts(j, 128)], id_sb)
                nc.vector.tensor_copy(va_sb[:, sig, 0:H], ptr)

        def emit_s(i, kind, g0):
            # scores matmuls + exp + mask for one group of tile i; returns pt
            if kind == "pair":
                ps = psum_s.tile([128, 1024], F32, tag="ps")
                for g in (0, 1):
                    sig = g0 + g
                    nc.tensor.matmul(ps[:, ts(g, 512)],
                                     kp_sb[ts(g, 64), ts(sig, 128)],
                                     qp_sb[ts(g, 64), ts(i, 512)],
                                     tile_position=(64 * g, 0),
                                     start=True, stop=True)
                pt = pt_pool.tile([128, 1024], FP)
                nc.scalar.activation(pt, ps,
                                     mybir.ActivationFunctionType.Exp,
                                     scale=SCALE)
                for g in (0, 1):
                    sig = g0 + g
                    if sig >= MASK_FROM[i]:
                        m = M_IDX[(i, sig)]
                        nc.vector.tensor_mul(pt[:, ts(g, 512)],
                                             pt[:, ts(g, 512)],
                                             masks_sb[:, m, :])
            else:  # singleton: full-width scores (PSUM zero-region rule),
                # but exp/mask/PV trimmed to the live cols [W0:512)
                sig = g0
                m = M_IDX[(i, sig)]
                ps = psum_s.tile([128, 512], F32, tag="ps")
                nc.tensor.matmul(ps,
                                 kp_sb[0:64, ts(sig, 128)],
                                 qp_sb[0:64, ts(i, 512)],
                                 tile_position=(0, 0), start=True, stop=True)
                pt = pt_pool.tile([128, 512], FP)
                nc.scalar.activation(pt[:, W0:512], ps[:, W0:512],
                                     mybir.ActivationFunctionType.Exp,
                                     scale=SCALE)
                nc.vector.tensor_mul(pt[:, W0:512], pt[:, W0:512],
                                     masks_sb[:, m, W0:512])
            return pt

        def groups_of(i):
            # leading full pairs, then (i<7) the nearly-dead singleton
            # (trimmed to cols [W0,512)), then the diagonal pair (2i, 2i+1)
            # last so the accumulation stop lands on a full-width matmul.
            cap = CAPS[i]
            groups = []
            lead = cap if i == 7 else 2 * i
            for g0 in range(0, lead, 2):
                groups.append(("pair", g0))
            if i == 0:
                # the first PV writing po must be full width (PSUM zero-region
                # start semantics), so the trimmed singleton goes last
                groups = [("pair", 0), ("single", 2)]
            elif i < 7:
                groups.append(("single", cap - 1))
                groups.append(("pair", 2 * i))
            return groups

        PEELED = {}                # tile -> pre-emitted leading score groups
        PEEL = 1                   # max lead pairs peeled across the boundary

        def attention(i, mid_q=None, mid=None, peel_next=False):
            po = psum_o.tile([H + 1, 512], F32, tag="oacc")
            groups = groups_of(i)

            state = {"first": True}

            def emit_pv(kind, g0, pt, last_grp):
                if kind == "pair":
                    for g in (0, 1):
                        sig = g0 + g
                        nc.tensor.matmul(po, va_sb[:, sig, :], pt[:, ts(g, 512)],
                                         start=state["first"],
                                         stop=(last_grp and g == 1))
                        state["first"] = False
                else:
                    nc.tensor.matmul(po[:, W0:512], va_sb[:, g0, :],
                                     pt[:, W0:512], start=state["first"],
                                     stop=last_grp)
                    state["first"] = False

            def finalize():
                osb = outsb_pool.tile([H + 1, 512], F32)
                nc.vector.tensor_copy(osb, po)
                # gpsimd SWDGE ring: keeps the sync HWDGE ring free for the
                # input stream (ring-full blocking serializes transfers)
                nc.gpsimd.dma_start(out=out[:, ts(i, 512)], in_=osb)

            # software pipeline: PV stream lags the scores stream by LAG
            # groups so the PE never stalls on the exp+mask latency; tail
            # PVs flush after mid() (the next tile's projections), by which
            # time their exps are long done.
            LAG = 3
            pend = []
            start_gi = 0
            if i in PEELED:        # leading groups emitted by the previous
                pend.extend(PEELED.pop(i))  # attention's peel
                start_gi = len(pend)
            for gi, (kind, g0) in enumerate(groups):
                if gi < start_gi:
                    continue
                if gi == len(groups) - 1 and mid_q is not None:
                    # next tile's q-projection BEFORE the last score group:
                    # its DVE evacuation then precedes this group's mask
                    # multiply in the DVE queue, so the peeled scores below
                    # aren't stalled behind an ACT-gated mask op
                    mid_q()
                    mid_q = None
                pt = emit_s(i, kind, g0)
                pend.append((kind, g0, pt))
                if len(pend) > LAG:
                    k_, g_, pt_ = pend.pop(0)
                    emit_pv(k_, g_, pt_, last_grp=False)
            if mid_q is not None:
                mid_q()
            if peel_next:
                # pre-emit the NEXT tile's leading score PAIRS so the ACT
                # engine rolls straight into them the moment this tile's exps
                # drain (instead of waiting for this tile's tail-PV flush or
                # the k/v projections in mid()).  Only lead pairs are safe to
                # peel: they need kp tiles projected attentions ago, while
                # the singleton/diag need the k-chunk projected in mid().
                nxt = groups_of(i + 1)
                n_lead = sum(1 for k_, g_ in nxt
                             if k_ == "pair" and g_ < 2 * (i + 1))
                PEELED[i + 1] = [(k_, g_, emit_s(i + 1, k_, g_))
                                 for k_, g_ in nxt[:min(PEEL, max(n_lead, 1))]]
            if mid is not None:
                # next tiles' k/v projections: the PE chews them while the
                # ACT engine runs this tile's trailing exps + the peel
                mid()
            for j, (k_, g_, pt_) in enumerate(pend):
                emit_pv(k_, g_, pt_, last_grp=(j == len(pend) - 1))
            finalize()

        # ---- compute schedule: deadline-aligned with the DMA stream ----
        # attention(i) consumes qp_i, kp s-tiles <= 2i+2, va s-tiles <= 2i+2.
        # Each attention's mid-hook carries upcoming projections so they
        # overlap that tile's trailing exp stream on the ACT engine.
        build_masks()
        k_proj(0)                  # s0-3
        q_proj(0)
        v_proj(0)                  # va 0-3
        attention(0, mid_q=lambda: q_proj(1),
                  mid=lambda: (k_proj(1), v_proj(1, 0, 2)), peel_next=True)
        attention(1, mid_q=lambda: q_proj(2),
                  mid=lambda: v_proj(1, 2, 4), peel_next=True)
        attention(2, mid_q=lambda: q_proj(3),
                  mid=lambda: (k_proj(2), v_proj(2, 0, 2)), peel_next=True)
        attention(3, mid_q=lambda: q_proj(4),
                  mid=lambda: v_proj(2, 2, 4), peel_next=True)
        attention(4, mid_q=lambda: q_proj(5),
                  mid=lambda: (k_proj(3), v_proj(3, 0, 2)), peel_next=True)
        attention(5, mid_q=lambda: q_proj(6),
                  mid=lambda: v_proj(3, 2, 4), peel_next=True)
        attention(6, mid_q=lambda: q_proj(7), peel_next=True)
        attention(7)

    nc.compile()
    return nc


_NC = None


def _get_nc():
    global _NC
    if _NC is None:
        _NC = build_bass()
    return _NC


def _prep_core_inputs(q, k, v, Wq, Wk, Wv):
    f2 = np.float16

    def wprep(W):
        # SBUF layout [p, ct, h] = W.T[ct*128+p, h]
        return np.ascontiguousarray(W.T.reshape(NCT, 128, H).transpose(1, 0, 2)).astype(f2)

    def xprep(x):
        # [p, chunk, ct, col] = x.T[128*ct+p, 512*chunk+col]
        xt = x.T.astype(f2)                       # [C, T']
        nch = xt.shape[1] // 512
        return np.ascontiguousarray(
            xt.reshape(NCT, 128, nch, 512).transpose(1, 2, 0, 3))

    w3_h = np.ascontiguousarray(
        np.stack([wprep(Wq), wprep(Wk), wprep(Wv)], axis=1))

    r = np.arange(128)
    in_maps = []
    for j in range(NCORES):
        b, p = j // 2, j % 2
        rows = (np.arange(TH) // 128) * 256 + p * 128 + (np.arange(TH) % 128)
        qT_h = xprep(q[b])
        kT_h = xprep(k[b][rows])
        vT_h = xprep(v[b][rows])
        misc_h = np.zeros((128, TQ + 64 + N_MASKED), f2)
        misc_h[:, 0:TQ] = np.arange(TQ, dtype=np.float32)[None, :]
        misc_h[0:64, TQ:TQ + 64] = np.eye(64, dtype=f2)
        for m in range(N_MASKED):
            misc_h[:, TQ + 64 + m] = (256 * m + 128 * p + r - 1).astype(f2)
        in_maps.append({
            "qT": qT_h, "kT": kT_h, "vT": vT_h,
            "w3": w3_h, "misc": misc_h,
        })
    return in_maps


def _run(inputs, trace=False, trace_kwargs=None):
    nc = _get_nc()
    in_maps = _prep_core_inputs(
        inputs["q"], inputs["k"], inputs["v"],
        inputs["Wq"], inputs["Wk"], inputs["Wv"])
    res = run_bass_kernel_spmd(nc, in_maps, list(range(NCORES)), trace=trace,
                               **(trace_kwargs or {}))
    outs = [res.results[j]["out"] for j in range(NCORES)]
    y = np.empty((B, T, H), np.float32)
    for b in range(B):
        s = outs[2 * b] + outs[2 * b + 1]      # [H+1, T]
        y[b] = (s[:H] / s[H:H + 1]).T
    return y, res


def kernel(q, k, v, Wq, Wk, Wv):
    y, _ = _run({"q": np.asarray(q), "k": np.asarray(k), "v": np.asarray(v),
                 "Wq": np.asarray(Wq), "Wk": np.asarray(Wk), "Wv": np.asarray(Wv)})
    return y

